# revision 7
# baseline (speedup 1.0000x reference)
"""Trainium2 Bass kernel for hierarchical-ODE + GRU sequence model (HNODE).

Strategy: pure data parallelism over the 64 independent (batch, person)
sequences -> 8 per NeuronCore.  On each core the recurrent state is kept
on-chip feature-major ([128-feature-chunk, 2, cols]) with columns ordered
joint-major (col = j*8 + seq).  All weights stay resident in SBUF.  Per
step: hierarchical ODE Euler updates (3 MLPs) then hierarchical GRU
updates (3 levels x 2 blocks), all as 128x128-blocked matmuls with the
weight stationary, plus a fused output projection.  Biases for GRU
block-0 ride a constant "ones" feature appended to the per-step x input;
other biases are applied with DVE broadcast-AP adds.

The time recurrence is sequential: a tc.For_i hardware loop over steps.
"""

import sys

for _p in ("/opt/trn_rl_repo", "/root/.axon_site/_ro/trn_rl_repo"):
    if _p not in sys.path:
        sys.path.insert(0, _p)

import numpy as np

import concourse.bass as bass
import concourse.tile as tile
from concourse import bacc, mybir
from concourse.bass_utils import run_bass_kernel_spmd

# ---- model dims (hardcoded from the problem spec) ----
B, S, P, J, DIN, DOUT, D, H = 16, 64, 4, 18, 2, 3, 256, 512
KSUB, DT = 2, 0.02
NCORES = 8
BPC = B // NCORES            # batches per core = 2
SEQ = BPC * P                # sequences per core = 8
C1, C2, C3 = 1 * SEQ, 5 * SEQ, 12 * SEQ      # group col counts 8, 40, 96
CT = J * SEQ                 # 144 total cols per core
DC = D // 128                # feature chunks of the hidden state = 2

F32 = mybir.dt.float32
BF16 = mybir.dt.bfloat16
AF = mybir.ActivationFunctionType
ALU = mybir.AluOpType


# ======================================================================
# host-side weight packing
# ======================================================================
class _Pack:
    """Packs [p, w] blocks into one [p, total] array, remembering offsets."""

    def __init__(self, p):
        self.p = p
        self.cols = 0
        self.chunks = []
        self.off = {}

    def add(self, key, arr):
        assert arr.shape[0] == self.p, (key, arr.shape)
        self.off[key] = self.cols
        self.chunks.append(np.ascontiguousarray(arr))
        self.cols += arr.shape[1]

    def finish(self, dtype):
        return np.ascontiguousarray(
            np.concatenate(self.chunks, axis=1).astype(dtype)
        )


def _blocks(Wmat):
    """[din, dout] -> [128, nk*nm*128], block index = k*nm + m."""
    din, dout = Wmat.shape
    nk, nm = din // 128, dout // 128
    cols = [Wmat[k * 128:(k + 1) * 128, m * 128:(m + 1) * 128]
            for k in range(nk) for m in range(nm)]
    return np.concatenate(cols, axis=1), nk, nm


def _bias_pack(b):
    """[n*128] -> [128, n] (chunk c in column c)."""
    n = b.shape[0] // 128
    return b.reshape(n, 128).T.copy()


def _host_pack(params, np_dtype):
    """Returns dict of numpy arrays + offset metadata for the builder."""
    w = _Pack(128)      # 128-row lhsT blocks (weights)
    wx = _Pack(3)       # 3-row lhsT blocks (x-part + ones/bias row)
    bz = _Pack(128)     # f32 bias packs
    meta = {}

    def addW(key, Wmat):
        blk, nk, nm = _blocks(np.asarray(Wmat, np.float32))
        w.add(key, blk)
        meta[key] = (nk, nm)

    # ODE MLPs.  Layer-3 weight+bias pre-scaled by the Euler factor.
    for name, pkey, scale in (("o1", "ODE_1", 2 * KSUB * DT),
                              ("o2", "ODE_2", KSUB * DT),
                              ("o3", "ODE_3", DT)):
        (W1, b1), (W2, b2), (W3, b3) = params[pkey]
        addW(name + "L1", W1)
        addW(name + "L2", W2)
        addW(name + "L3", np.asarray(W3, np.float32) * scale)
        bz.add(name + "b1", _bias_pack(np.asarray(b1, np.float32)))
        bz.add(name + "b2", _bias_pack(np.asarray(b2, np.float32)))
        bz.add(name + "b3", _bias_pack(np.asarray(b3, np.float32) * scale))

    # GRUs.  gates order (r, z, n) along 3*D.
    for lvl, pkey in ((1, "GRU_1"), (2, "GRU_2"), (3, "GRU_3")):
        blk0, blk1 = params[pkey]
        Wi0 = np.asarray(blk0["Wi"], np.float32)     # [din0, 768]
        Wh0 = np.asarray(blk0["Wh"], np.float32)
        bi0 = np.asarray(blk0["bi"], np.float32)
        bh0 = np.asarray(blk0["bh"], np.float32)
        # block0 x-part (+ones bias row): lvl1 din=2 (x only);
        # lvl2/3 din=258 = [parent 256 ; x 2]
        if lvl == 1:
            Wxp = Wi0                                # [2, 768]
        else:
            addW(f"g{lvl}p0", Wi0[:D])               # parent part [256, 768]
            Wxp = Wi0[D:]                            # [2, 768]
        ones_row = np.concatenate([bi0[:2 * D] + bh0[:2 * D], bi0[2 * D:]])
        wxa = np.concatenate([Wxp, ones_row[None, :]], axis=0)   # [3, 768]
        wx.add(f"g{lvl}x0",
               np.concatenate([wxa[:, m * 128:(m + 1) * 128] for m in range(6)],
                              axis=1))
        addW(f"g{lvl}h0", Wh0)
        bz.add(f"g{lvl}bhn0", _bias_pack(bh0[2 * D:]))

        Wi1 = np.asarray(blk1["Wi"], np.float32)
        Wh1 = np.asarray(blk1["Wh"], np.float32)
        bi1 = np.asarray(blk1["bi"], np.float32)
        bh1 = np.asarray(blk1["bh"], np.float32)
        addW(f"g{lvl}i1", Wi1)
        addW(f"g{lvl}h1", Wh1)
        bz.add(f"g{lvl}brz1", _bias_pack(bi1[:2 * D] + bh1[:2 * D]))
        bz.add(f"g{lvl}bin1", _bias_pack(bi1[2 * D:]))
        bz.add(f"g{lvl}bhn1", _bias_pack(bh1[2 * D:]))

    Wo, bo = params["out"]
    Wo = np.asarray(Wo, np.float32)                  # [256, 3]
    wout = np.concatenate([Wo[k * 128:(k + 1) * 128] for k in range(2)],
                          axis=1)                    # [128, 6]

    h0 = np.asarray(params["h0"], np.float32).reshape(D)     # [256]
    hinit = np.broadcast_to(h0.reshape(2, 128, 1), (2, 128, CT))
    hinit = np.ascontiguousarray(hinit.transpose(1, 0, 2)).astype(np_dtype)

    arrays = {
        "wmat": w.finish(np_dtype),
        "wx": wx.finish(np_dtype),
        "wout": np.ascontiguousarray(wout.astype(np_dtype)),
        "bias": bz.finish(np.float32),
        "bo": np.asarray(bo, np.float32).reshape(3, 1).copy(),
        "hinit": hinit,
    }
    offs = {"w": w.off, "wx": wx.off, "b": bz.off, "meta": meta}
    return arrays, offs


# ======================================================================
# device kernel builder
# ======================================================================
def _build(steps, offs, adt, use_loop=None):
    """Builds the Bass module. adt = matmul/activation dtype."""
    meta = offs["meta"]
    nc = bacc.Bacc("TRN2", target_bir_lowering=False, debug=False)

    xs_d = nc.dram_tensor("xs", (steps, 3, CT), adt, kind="ExternalInput").ap()
    wmat_d = nc.dram_tensor("wmat", (128, offs["_wcols"]), adt,
                            kind="ExternalInput").ap()
    wx_d = nc.dram_tensor("wx", (3, offs["_wxcols"]), adt,
                          kind="ExternalInput").ap()
    wout_d = nc.dram_tensor("wout", (128, 6), adt, kind="ExternalInput").ap()
    bias_d = nc.dram_tensor("bias", (128, offs["_bcols"]), F32,
                            kind="ExternalInput").ap()
    bo_d = nc.dram_tensor("bo", (3, 1), F32, kind="ExternalInput").ap()
    hin_d = nc.dram_tensor("hinit", (128, 2, CT), adt,
                           kind="ExternalInput").ap()
    out_d = nc.dram_tensor("out", (steps, 3, CT), F32,
                           kind="ExternalOutput").ap()

    with tile.TileContext(nc) as tc:
        from contextlib import ExitStack
        with ExitStack() as ctx:
            persist = ctx.enter_context(tc.tile_pool(name="persist", bufs=1))
            xp = ctx.enter_context(tc.tile_pool(name="xp", bufs=2))
            sbp = ctx.enter_context(tc.tile_pool(name="sbp", bufs=3))
            psp1 = ctx.enter_context(
                tc.tile_pool(name="psp1", bufs=3, space="PSUM"))
            psp2 = ctx.enter_context(
                tc.tile_pool(name="psp2", bufs=2, space="PSUM"))
            psp3 = ctx.enter_context(
                tc.tile_pool(name="psp3", bufs=2, space="PSUM"))
            psp4 = ctx.enter_context(
                tc.tile_pool(name="psp4", bufs=1, space="PSUM"))
            outp = ctx.enter_context(tc.tile_pool(name="outp", bufs=2))

            # ---- resident tensors
            w_s = persist.tile([128, offs["_wcols"]], adt)
            CH = 7808  # dma chunk cols
            for c0 in range(0, offs["_wcols"], CH):
                c1 = min(c0 + CH, offs["_wcols"])
                nc.sync.dma_start(out=w_s[:, c0:c1], in_=wmat_d[:, c0:c1])
            wx_s = persist.tile([3, offs["_wxcols"]], adt)
            nc.sync.dma_start(out=wx_s, in_=wx_d)
            wo_s = persist.tile([128, 6], adt)
            nc.sync.dma_start(out=wo_s, in_=wout_d)
            b_s = persist.tile([128, offs["_bcols"]], F32)
            nc.sync.dma_start(out=b_s, in_=bias_d)
            bo_s = persist.tile([3, 1], F32)
            nc.sync.dma_start(out=bo_s, in_=bo_d)

            h1 = persist.tile([128, 2, C1], adt)
            h2 = persist.tile([128, 2, C2], adt)
            h3 = persist.tile([128, 2, C3], adt)
            nc.sync.dma_start(out=h1, in_=hin_d[:, :, 0:C1])
            nc.sync.dma_start(out=h2, in_=hin_d[:, :, C1:C1 + C2])
            nc.sync.dma_start(out=h3, in_=hin_d[:, :, C1 + C2:CT])

            # ---- AP helpers
            def wblk(key, k, m):
                nm = meta[key][1]
                o = offs["w"][key] + (k * nm + m) * 128
                return w_s[:, o:o + 128]

            def wxblk(key, m):
                o = offs["wx"][key] + m * 128
                return wx_s[:, o:o + 128]

            def bias_bc(key, nm, r):
                o = offs["b"][key]
                sl = b_s[:, o:o + nm]
                return bass.AP(tensor=sl.tensor, offset=sl.offset,
                               ap=[sl.ap[0], sl.ap[-1], [0, r]])

            def bias_col(key, c):
                o = offs["b"][key]
                return b_s[:, o + c:o + c + 1]

            def bc5(ht, c):
                # [128, 8] group-1 chunk -> broadcast to 5 joints
                src = ht[:, c, :]
                return bass.AP(tensor=src.tensor, offset=src.offset,
                               ap=[src.ap[0], [0, 5], [1, SEQ]])

            def bc_chain(ht, c):
                # h2 joints 2..5 (cols 8..40) each broadcast to its 3 children
                src = ht[:, c, SEQ:5 * SEQ]
                return bass.AP(tensor=src.tensor, offset=src.offset,
                               ap=[src.ap[0], [SEQ, 4], [0, 3], [1, SEQ]])

            def mm(ps_slice, lhsT, rhs, first, last):
                nc.tensor.matmul(ps_slice, lhsT=lhsT, rhs=rhs,
                                 start=first, stop=last)

            # ---- dense tanh layer: out[:,m,:] = tanh(sum_k Wk^T srck + b)
            def dense_tanh(key, bkey, srcs, r):
                nm = meta[key][1]
                ps = psp1.tile([128, nm, r], F32, tag="ps")
                nsrc = len(srcs)
                for m in range(nm):
                    for ki, src in enumerate(srcs):
                        mm(ps[:, m, :], wblk(key, ki, m), src,
                           ki == 0, ki == nsrc - 1)
                st = sbp.tile([128, nm, r], F32, tag="st")
                nc.vector.tensor_tensor(out=st, in0=ps,
                                        in1=bias_bc(bkey, nm, r), op=ALU.add)
                a = sbp.tile([128, nm, r], adt, tag="act")
                nc.scalar.activation(out=a, in_=st, func=AF.Tanh)
                return a

            def dense_update(key, bkey, srcs, ht, r):
                # ht[:,c,:] += (sum_k Wk^T srck)[:,c,:] + b_c   (scaled W,b)
                nm = meta[key][1]
                ps = psp1.tile([128, nm, r], F32, tag="ps")
                nsrc = len(srcs)
                for m in range(nm):
                    for ki, src in enumerate(srcs):
                        mm(ps[:, m, :], wblk(key, ki, m), src,
                           ki == 0, ki == nsrc - 1)
                for c in range(DC):
                    nc.vector.scalar_tensor_tensor(
                        out=ht[:, c, :], in0=ps[:, c, :],
                        scalar=bias_col(bkey, c), in1=ht[:, c, :],
                        op0=ALU.add, op1=ALU.add)

            def ode_mlp(name, srcs1, ht, r):
                a1 = dense_tanh(name + "L1", name + "b1", srcs1, r)
                a2 = dense_tanh(name + "L2", name + "b2",
                                [a1[:, k, :] for k in range(4)], r)
                dense_update(name + "L3", name + "b3",
                             [a2[:, k, :] for k in range(4)], ht, r)

            # ---- GRU block.
            # rz_srcs: list of (rhs, lhsT_fn(m_abs)) accumulated for gates r,z
            #          (m 0..3) AND for gi_n (m 4..5) -- the x/input part.
            # h_srcs:  same for the hidden part (Wh) -> rz psum and ghn psum.
            def gru_block(rz_srcs, h_srcs, ht, r, bkeys, biased_by_ones):
                ps_rz = psp2.tile([128, 4, r], F32, tag="rz")
                allsrc = rz_srcs + h_srcs
                na = len(allsrc)
                for m in range(4):
                    for ki, (rhs, lf) in enumerate(allsrc):
                        mm(ps_rz[:, m, :], lf(m), rhs, ki == 0, ki == na - 1)
                ps_n = psp3.tile([128, 2, r], F32, tag="nh")
                ni = len(rz_srcs)
                for m in range(2):
                    for ki, (rhs, lf) in enumerate(rz_srcs):
                        mm(ps_n[:, m, :], lf(m + 4), rhs, ki == 0, ki == ni - 1)
                ps_hn = psp3.tile([128, 2, r], F32, tag="nh")
                nh = len(h_srcs)
                for m in range(2):
                    for ki, (rhs, lf) in enumerate(h_srcs):
                        mm(ps_hn[:, m, :], lf(m + 4), rhs, ki == 0,
                           ki == nh - 1)

                if biased_by_ones:
                    rz_in = ps_rz
                else:
                    rz_in = sbp.tile([128, 4, r], F32, tag="st")
                    nc.vector.tensor_tensor(out=rz_in, in0=ps_rz,
                                            in1=bias_bc(bkeys["brz"], 4, r),
                                            op=ALU.add)
                rz = sbp.tile([128, 4, r], adt, tag="rz")
                nc.scalar.activation(out=rz, in_=rz_in, func=AF.Sigmoid)

                t2 = sbp.tile([128, 2, r], F32, tag="t2")
                for c in range(2):
                    nc.vector.scalar_tensor_tensor(
                        out=t2[:, c, :], in0=ps_hn[:, c, :],
                        scalar=bias_col(bkeys["bhn"], c), in1=rz[:, c, :],
                        op0=ALU.add, op1=ALU.mult)
                s_n = sbp.tile([128, 2, r], F32, tag="t2")
                if biased_by_ones:
                    nc.vector.tensor_tensor(out=s_n, in0=ps_n, in1=t2,
                                            op=ALU.add)
                else:
                    for c in range(2):
                        nc.vector.scalar_tensor_tensor(
                            out=s_n[:, c, :], in0=ps_n[:, c, :],
                            scalar=bias_col(bkeys["bin"], c), in1=t2[:, c, :],
                            op0=ALU.add, op1=ALU.add)
                n_t = sbp.tile([128, 2, r], adt, tag="nt")
                nc.scalar.activation(out=n_t, in_=s_n, func=AF.Tanh)

                d = sbp.tile([128, 2, r], F32, tag="de")
                nc.vector.tensor_tensor(out=d, in0=ht, in1=n_t,
                                        op=ALU.subtract)
                e = sbp.tile([128, 2, r], F32, tag="de")
                nc.vector.tensor_tensor(out=e, in0=rz[:, 2:4, :], in1=d,
                                        op=ALU.mult)
                return n_t, e    # h' = n + e, written by caller

            def h_srcs_of(key, ht):
                return [(ht[:, k, :], (lambda kk: (lambda m: wblk(key, kk, m)))(k))
                        for k in range(DC)]

            def gru_level(lvl, ht, r, x_ap, par_srcs):
                # block 0
                rz_srcs = list(par_srcs)
                rz_srcs.append((x_ap, lambda m: wxblk(f"g{lvl}x0", m)))
                g_out = sbp.tile([128, 2, r], adt, tag="nt")
                n_t, e = gru_block(rz_srcs, h_srcs_of(f"g{lvl}h0", ht), ht, r,
                                   {"bhn": f"g{lvl}bhn0"}, True)
                nc.vector.tensor_tensor(out=g_out, in0=n_t, in1=e, op=ALU.add)
                # block 1 (writes state in place)
                n_t, e = gru_block(h_srcs_of(f"g{lvl}i1", g_out),
                                   h_srcs_of(f"g{lvl}h1", ht), ht, r,
                                   {"brz": f"g{lvl}brz1",
                                    "bin": f"g{lvl}bin1",
                                    "bhn": f"g{lvl}bhn1"}, False)
                nc.vector.tensor_tensor(out=ht, in0=n_t, in1=e, op=ALU.add)

            # ---- one time step
            def step_body(i):
                x_t = xp.tile([3, CT], adt)
                nc.sync.dma_start(out=x_t, in_=xs_d[i])

                # ODE phase
                ode_mlp("o1", [h1[:, k, :] for k in range(DC)], h1, C1)
                for _i in range(KSUB):
                    ode_mlp("o2",
                            [bc5(h1, 0), bc5(h1, 1),
                             h2[:, 0, :], h2[:, 1, :]], h2, C2)
                    for _j in range(KSUB):
                        ode_mlp("o3",
                                [bc_chain(h2, 0), bc_chain(h2, 1),
                                 h3[:, 0, :], h3[:, 1, :]], h3, C3)

                # GRU phase
                gru_level(1, h1, C1, x_t[:, 0:C1], [])
                par2 = [(bc5(h1, k),
                         (lambda kk: (lambda m: wblk("g2p0", kk, m)))(k))
                        for k in range(DC)]
                gru_level(2, h2, C2, x_t[:, C1:C1 + C2], par2)
                par3 = [(bc_chain(h2, k),
                         (lambda kk: (lambda m: wblk("g3p0", kk, m)))(k))
                        for k in range(DC)]
                gru_level(3, h3, C3, x_t[:, C1 + C2:CT], par3)

                # output projection
                ps_o = psp4.tile([3, CT], F32, tag="po")
                for c0, r, ht in ((0, C1, h1), (C1, C2, h2), (C1 + C2, C3, h3)):
                    for k in range(DC):
                        mm(ps_o[:, c0:c0 + r], wo_s[:, k * 3:(k + 1) * 3],
                           ht[:, k, :], k == 0, k == DC - 1)
                o_t = outp.tile([3, CT], F32)
                nc.vector.tensor_scalar_add(out=o_t, in0=ps_o,
                                            scalar1=bo_s[:, 0:1])
                nc.sync.dma_start(out=out_d[i], in_=o_t)

            if use_loop is None:
                use_loop = steps > 4
            if not use_loop:
                for i in range(steps):
                    step_body(i)
            else:
                with tc.For_i(0, steps,
                              hint_engines=(mybir.EngineType.PE,
                                            mybir.EngineType.DVE,
                                            mybir.EngineType.Activation)) as i:
                    step_body(i)

    nc.compile()
    return nc


# ======================================================================
# host entry point
# ======================================================================
_CACHE = {}
LAST_EXEC_NS = None


def _get_module(steps, offs_key, offs, adt):
    key = (steps, offs_key, str(adt))
    if key not in _CACHE:
        _CACHE[key] = _build(steps, offs, adt)
    return _CACHE[key]


def _prep_core_inputs(x2d, mask, arrays, np_dtype, steps):
    """Per-core xs arrays: [steps, 3, CT], feats (x0, x1, 1), col=j*SEQ+s."""
    xm = (np.asarray(x2d, np.float32) * np.asarray(mask, np.float32))
    # [B, S, P, J, 2] -> per core [S, 2, J, BPC, P] -> [S, 2, J*SEQ]
    in_maps = []
    for c in range(NCORES):
        xc = xm[c * BPC:(c + 1) * BPC, :steps]          # [BPC, S, P, J, 2]
        xc = xc.transpose(1, 4, 3, 0, 2)                # [S, 2, J, BPC, P]
        xc = xc.reshape(steps, 2, CT)
        xs = np.empty((steps, 3, CT), np.float32)
        xs[:, :2] = xc
        xs[:, 2] = 1.0
        m = {"xs": xs.astype(np_dtype)}
        m.update(arrays)
        in_maps.append(m)
    return in_maps


def _assemble_output(results, steps):
    """results[c]['out'] [steps, 3, CT] -> full (B, S, P, J, 3)."""
    out = np.empty((B, steps, P, J, DOUT), np.float32)
    for c in range(NCORES):
        oc = results[c]["out"].reshape(steps, DOUT, J, BPC, P)
        out[c * BPC:(c + 1) * BPC] = oc.transpose(3, 0, 4, 2, 1)
    return out


def kernel(ts, mask, x2d, g1, g2, g3, params, _steps=S, _adt=F32,
           _trace=False):
    np_dtype = np.float32 if _adt == F32 else mybir.dt.np(_adt)
    arrays, offs = _host_pack(params, np_dtype)
    offs["_wcols"] = arrays["wmat"].shape[1]
    offs["_wxcols"] = arrays["wx"].shape[1]
    offs["_bcols"] = arrays["bias"].shape[1]
    nc = _get_module(_steps, "v1", offs, _adt)
    in_maps = _prep_core_inputs(x2d, mask, arrays, np_dtype, _steps)
    res = run_bass_kernel_spmd(nc, in_maps, core_ids=list(range(NCORES)),
                               trace=_trace)
    out = _assemble_output(res.results, _steps)
    global LAST_EXEC_NS
    LAST_EXEC_NS = res.exec_time_ns
    return out


# revision 9
# speedup vs baseline: 293.9793x; 293.9793x over previous
"""Trainium2 Bass kernel for hierarchical-ODE + GRU sequence model (HNODE).

Strategy: pure data parallelism over the 64 independent (batch, person)
sequences -> 8 per NeuronCore.  On each core the recurrent state is kept
on-chip feature-major ([128-feature-chunk, 2, cols]) with columns ordered
joint-major (col = j*8 + seq).  All weights stay resident in SBUF.  Per
step: hierarchical ODE Euler updates (3 MLPs) then hierarchical GRU
updates (3 levels x 2 blocks), all as 128x128-blocked matmuls with the
weight stationary, plus a fused output projection.  Biases for GRU
block-0 ride a constant "ones" feature appended to the per-step x input;
other biases are applied with DVE broadcast-AP adds.

The time recurrence is sequential: a tc.For_i hardware loop over steps.
"""

import sys

for _p in ("/opt/trn_rl_repo", "/root/.axon_site/_ro/trn_rl_repo"):
    if _p not in sys.path:
        sys.path.insert(0, _p)

import numpy as np

import concourse.bass as bass
import concourse.tile as tile
from concourse import bacc, mybir
from concourse.bass_utils import run_bass_kernel_spmd

# ---- model dims (hardcoded from the problem spec) ----
B, S, P, J, DIN, DOUT, D, H = 16, 64, 4, 18, 2, 3, 256, 512
KSUB, DT = 2, 0.02
NCORES = 8
BPC = B // NCORES            # batches per core = 2
SEQ = BPC * P                # sequences per core = 8
C1, C2, C3 = 1 * SEQ, 5 * SEQ, 12 * SEQ      # group col counts 8, 40, 96
CT = J * SEQ                 # 144 total cols per core
DC = D // 128                # feature chunks of the hidden state = 2

F32 = mybir.dt.float32
BF16 = mybir.dt.bfloat16
AF = mybir.ActivationFunctionType
ALU = mybir.AluOpType


# ======================================================================
# host-side weight packing
# ======================================================================
class _Pack:
    """Packs [p, w] blocks into one [p, total] array, remembering offsets."""

    def __init__(self, p):
        self.p = p
        self.cols = 0
        self.chunks = []
        self.off = {}

    def add(self, key, arr):
        assert arr.shape[0] == self.p, (key, arr.shape)
        self.off[key] = self.cols
        self.chunks.append(np.ascontiguousarray(arr))
        self.cols += arr.shape[1]

    def finish(self, dtype):
        return np.ascontiguousarray(
            np.concatenate(self.chunks, axis=1).astype(dtype)
        )


def _blocks(Wmat):
    """[din, dout] -> [128, nk*nm*128], block index = k*nm + m."""
    din, dout = Wmat.shape
    nk, nm = din // 128, dout // 128
    cols = [Wmat[k * 128:(k + 1) * 128, m * 128:(m + 1) * 128]
            for k in range(nk) for m in range(nm)]
    return np.concatenate(cols, axis=1), nk, nm


def _bias_pack(b):
    """[n*128] -> [128, n] (chunk c in column c)."""
    n = b.shape[0] // 128
    return b.reshape(n, 128).T.copy()


def _host_pack(params, np_dtype):
    """Returns dict of numpy arrays + offset metadata for the builder."""
    w = _Pack(128)      # 128-row lhsT blocks (weights)
    wx = _Pack(3)       # 3-row lhsT blocks (x-part + ones/bias row)
    bz = _Pack(128)     # f32 bias packs
    meta = {}

    def addW(key, Wmat):
        blk, nk, nm = _blocks(np.asarray(Wmat, np.float32))
        w.add(key, blk)
        meta[key] = (nk, nm)

    # ODE MLPs.  Layer-3 weight+bias pre-scaled by the Euler factor.
    for name, pkey, scale in (("o1", "ODE_1", 2 * KSUB * DT),
                              ("o2", "ODE_2", KSUB * DT),
                              ("o3", "ODE_3", DT)):
        (W1, b1), (W2, b2), (W3, b3) = params[pkey]
        addW(name + "L1", W1)
        addW(name + "L2", W2)
        addW(name + "L3", np.asarray(W3, np.float32) * scale)
        bz.add(name + "b1", _bias_pack(np.asarray(b1, np.float32)))
        bz.add(name + "b2", _bias_pack(np.asarray(b2, np.float32)))
        bz.add(name + "b3", _bias_pack(np.asarray(b3, np.float32) * scale))

    # GRUs.  gates order (r, z, n) along 3*D.
    for lvl, pkey in ((1, "GRU_1"), (2, "GRU_2"), (3, "GRU_3")):
        blk0, blk1 = params[pkey]
        Wi0 = np.asarray(blk0["Wi"], np.float32)     # [din0, 768]
        Wh0 = np.asarray(blk0["Wh"], np.float32)
        bi0 = np.asarray(blk0["bi"], np.float32)
        bh0 = np.asarray(blk0["bh"], np.float32)
        # block0 x-part (+ones bias row): lvl1 din=2 (x only);
        # lvl2/3 din=258 = [parent 256 ; x 2]
        if lvl == 1:
            Wxp = Wi0                                # [2, 768]
        else:
            addW(f"g{lvl}p0", Wi0[:D])               # parent part [256, 768]
            Wxp = Wi0[D:]                            # [2, 768]
        ones_row = np.concatenate([bi0[:2 * D] + bh0[:2 * D], bi0[2 * D:]])
        wxa = np.concatenate([Wxp, ones_row[None, :]], axis=0)   # [3, 768]
        wx.add(f"g{lvl}x0",
               np.concatenate([wxa[:, m * 128:(m + 1) * 128] for m in range(6)],
                              axis=1))
        addW(f"g{lvl}h0", Wh0)
        bz.add(f"g{lvl}bhn0", _bias_pack(bh0[2 * D:]))

        Wi1 = np.asarray(blk1["Wi"], np.float32)
        Wh1 = np.asarray(blk1["Wh"], np.float32)
        bi1 = np.asarray(blk1["bi"], np.float32)
        bh1 = np.asarray(blk1["bh"], np.float32)
        addW(f"g{lvl}i1", Wi1)
        addW(f"g{lvl}h1", Wh1)
        bz.add(f"g{lvl}brz1", _bias_pack(bi1[:2 * D] + bh1[:2 * D]))
        bz.add(f"g{lvl}bin1", _bias_pack(bi1[2 * D:]))
        bz.add(f"g{lvl}bhn1", _bias_pack(bh1[2 * D:]))

    Wo, bo = params["out"]
    Wo = np.asarray(Wo, np.float32)                  # [256, 3]
    wout = np.concatenate([Wo[k * 128:(k + 1) * 128] for k in range(2)],
                          axis=1)                    # [128, 6]

    h0 = np.asarray(params["h0"], np.float32).reshape(D)     # [256]
    hinit = np.broadcast_to(h0.reshape(2, 128, 1), (2, 128, CT))
    hinit = np.ascontiguousarray(hinit.transpose(1, 0, 2)).astype(np_dtype)

    arrays = {
        "wmat": w.finish(np_dtype),
        "wx": wx.finish(np_dtype),
        "wout": np.ascontiguousarray(wout.astype(np_dtype)),
        "bias": bz.finish(np.float32),
        "bo": np.asarray(bo, np.float32).reshape(3, 1).copy(),
        "hinit": hinit,
    }
    offs = {"w": w.off, "wx": wx.off, "b": bz.off, "meta": meta}
    return arrays, offs


# ======================================================================
# device kernel builder
# ======================================================================
def _build(steps, offs, adt, use_loop=None, rep=1):
    """Builds the Bass module. adt = matmul/activation dtype."""
    meta = offs["meta"]
    nc = bacc.Bacc("TRN2", target_bir_lowering=False, debug=False)

    xs_d = nc.dram_tensor("xs", (steps, 3, CT), adt, kind="ExternalInput").ap()
    wmat_d = nc.dram_tensor("wmat", (128, offs["_wcols"]), adt,
                            kind="ExternalInput").ap()
    wx_d = nc.dram_tensor("wx", (3, offs["_wxcols"]), adt,
                          kind="ExternalInput").ap()
    wout_d = nc.dram_tensor("wout", (128, 6), adt, kind="ExternalInput").ap()
    bias_d = nc.dram_tensor("bias", (128, offs["_bcols"]), F32,
                            kind="ExternalInput").ap()
    bo_d = nc.dram_tensor("bo", (3, 1), F32, kind="ExternalInput").ap()
    hin_d = nc.dram_tensor("hinit", (128, 2, CT), adt,
                           kind="ExternalInput").ap()
    out_d = nc.dram_tensor("out", (steps, 3, CT), F32,
                           kind="ExternalOutput").ap()

    with tile.TileContext(nc) as tc:
        from contextlib import ExitStack
        with ExitStack() as ctx:
            persist = ctx.enter_context(tc.tile_pool(name="persist", bufs=1))
            xp = ctx.enter_context(tc.tile_pool(name="xp", bufs=2))
            sbp = ctx.enter_context(tc.tile_pool(name="sbp", bufs=3))
            psp1 = ctx.enter_context(
                tc.tile_pool(name="psp1", bufs=3, space="PSUM"))
            psp2 = ctx.enter_context(
                tc.tile_pool(name="psp2", bufs=2, space="PSUM"))
            psp3 = ctx.enter_context(
                tc.tile_pool(name="psp3", bufs=2, space="PSUM"))
            psp4 = ctx.enter_context(
                tc.tile_pool(name="psp4", bufs=1, space="PSUM"))
            outp = ctx.enter_context(tc.tile_pool(name="outp", bufs=2))

            # ---- resident tensors
            w_s = persist.tile([128, offs["_wcols"]], adt)
            CH = 7808  # dma chunk cols
            for c0 in range(0, offs["_wcols"], CH):
                c1 = min(c0 + CH, offs["_wcols"])
                nc.sync.dma_start(out=w_s[:, c0:c1], in_=wmat_d[:, c0:c1])
            wx_s = persist.tile([3, offs["_wxcols"]], adt)
            nc.sync.dma_start(out=wx_s, in_=wx_d)
            wo_s = persist.tile([128, 6], adt)
            nc.sync.dma_start(out=wo_s, in_=wout_d)
            b_s = persist.tile([128, offs["_bcols"]], F32)
            nc.sync.dma_start(out=b_s, in_=bias_d)
            bo_s = persist.tile([3, 1], F32)
            nc.sync.dma_start(out=bo_s, in_=bo_d)

            h1 = persist.tile([128, 2, C1], adt)
            h2 = persist.tile([128, 2, C2], adt)
            h3 = persist.tile([128, 2, C3], adt)
            nc.sync.dma_start(out=h1, in_=hin_d[:, :, 0:C1])
            nc.sync.dma_start(out=h2, in_=hin_d[:, :, C1:C1 + C2])
            nc.sync.dma_start(out=h3, in_=hin_d[:, :, C1 + C2:CT])

            # ---- AP helpers
            def wblk(key, k, m):
                nm = meta[key][1]
                o = offs["w"][key] + (k * nm + m) * 128
                return w_s[:, o:o + 128]

            def wxblk(key, m):
                o = offs["wx"][key] + m * 128
                return wx_s[:, o:o + 128]

            def bias_bc(key, nm, r):
                o = offs["b"][key]
                sl = b_s[:, o:o + nm]
                return bass.AP(tensor=sl.tensor, offset=sl.offset,
                               ap=[sl.ap[0], sl.ap[-1], [0, r]])

            def bias_col(key, c):
                o = offs["b"][key]
                return b_s[:, o + c:o + c + 1]

            def bc5(ht, c):
                # [128, 8] group-1 chunk -> broadcast to 5 joints
                src = ht[:, c, :]
                return bass.AP(tensor=src.tensor, offset=src.offset,
                               ap=[src.ap[0], [0, 5], [1, SEQ]])

            def bc_chain(ht, c):
                # h2 joints 2..5 (cols 8..40) each broadcast to its 3 children
                src = ht[:, c, SEQ:5 * SEQ]
                return bass.AP(tensor=src.tensor, offset=src.offset,
                               ap=[src.ap[0], [SEQ, 4], [0, 3], [1, SEQ]])

            def mm(ps_slice, lhsT, rhs, first, last):
                nc.tensor.matmul(ps_slice, lhsT=lhsT, rhs=rhs,
                                 start=first, stop=last)

            # ---- dense tanh layer: out[:,m,:] = tanh(sum_k Wk^T srck + b)
            def dense_tanh(key, bkey, srcs, r):
                nm = meta[key][1]
                ps = psp1.tile([128, nm, r], F32, tag="ps")
                nsrc = len(srcs)
                for m in range(nm):
                    for ki, src in enumerate(srcs):
                        mm(ps[:, m, :], wblk(key, ki, m), src,
                           ki == 0, ki == nsrc - 1)
                st = sbp.tile([128, nm, r], F32, tag="st")
                nc.vector.tensor_tensor(out=st, in0=ps,
                                        in1=bias_bc(bkey, nm, r), op=ALU.add)
                a = sbp.tile([128, nm, r], adt, tag="act")
                nc.scalar.activation(out=a, in_=st, func=AF.Tanh)
                return a

            def dense_update(key, bkey, srcs, ht, r):
                # ht[:,c,:] += (sum_k Wk^T srck)[:,c,:] + b_c   (scaled W,b)
                nm = meta[key][1]
                ps = psp1.tile([128, nm, r], F32, tag="ps")
                nsrc = len(srcs)
                for m in range(nm):
                    for ki, src in enumerate(srcs):
                        mm(ps[:, m, :], wblk(key, ki, m), src,
                           ki == 0, ki == nsrc - 1)
                for c in range(DC):
                    nc.vector.scalar_tensor_tensor(
                        out=ht[:, c, :], in0=ps[:, c, :],
                        scalar=bias_col(bkey, c), in1=ht[:, c, :],
                        op0=ALU.add, op1=ALU.add)

            def ode_mlp(name, srcs1, ht, r):
                a1 = dense_tanh(name + "L1", name + "b1", srcs1, r)
                a2 = dense_tanh(name + "L2", name + "b2",
                                [a1[:, k, :] for k in range(4)], r)
                dense_update(name + "L3", name + "b3",
                             [a2[:, k, :] for k in range(4)], ht, r)

            # ---- GRU block.
            # rz_srcs: list of (rhs, lhsT_fn(m_abs)) accumulated for gates r,z
            #          (m 0..3) AND for gi_n (m 4..5) -- the x/input part.
            # h_srcs:  same for the hidden part (Wh) -> rz psum and ghn psum.
            def gru_block(rz_srcs, h_srcs, ht, r, bkeys, biased_by_ones):
                ps_rz = psp2.tile([128, 4, r], F32, tag="rz")
                allsrc = rz_srcs + h_srcs
                na = len(allsrc)
                for m in range(4):
                    for ki, (rhs, lf) in enumerate(allsrc):
                        mm(ps_rz[:, m, :], lf(m), rhs, ki == 0, ki == na - 1)
                ps_n = psp3.tile([128, 2, r], F32, tag="nh")
                ni = len(rz_srcs)
                for m in range(2):
                    for ki, (rhs, lf) in enumerate(rz_srcs):
                        mm(ps_n[:, m, :], lf(m + 4), rhs, ki == 0, ki == ni - 1)
                ps_hn = psp3.tile([128, 2, r], F32, tag="nh")
                nh = len(h_srcs)
                for m in range(2):
                    for ki, (rhs, lf) in enumerate(h_srcs):
                        mm(ps_hn[:, m, :], lf(m + 4), rhs, ki == 0,
                           ki == nh - 1)

                if biased_by_ones:
                    rz_in = ps_rz
                else:
                    rz_in = sbp.tile([128, 4, r], F32, tag="st")
                    nc.vector.tensor_tensor(out=rz_in, in0=ps_rz,
                                            in1=bias_bc(bkeys["brz"], 4, r),
                                            op=ALU.add)
                rz = sbp.tile([128, 4, r], adt, tag="rz")
                nc.scalar.activation(out=rz, in_=rz_in, func=AF.Sigmoid)

                t2 = sbp.tile([128, 2, r], F32, tag="t2")
                for c in range(2):
                    nc.vector.scalar_tensor_tensor(
                        out=t2[:, c, :], in0=ps_hn[:, c, :],
                        scalar=bias_col(bkeys["bhn"], c), in1=rz[:, c, :],
                        op0=ALU.add, op1=ALU.mult)
                s_n = sbp.tile([128, 2, r], F32, tag="t2")
                if biased_by_ones:
                    nc.vector.tensor_tensor(out=s_n, in0=ps_n, in1=t2,
                                            op=ALU.add)
                else:
                    for c in range(2):
                        nc.vector.scalar_tensor_tensor(
                            out=s_n[:, c, :], in0=ps_n[:, c, :],
                            scalar=bias_col(bkeys["bin"], c), in1=t2[:, c, :],
                            op0=ALU.add, op1=ALU.add)
                n_t = sbp.tile([128, 2, r], adt, tag="nt")
                nc.scalar.activation(out=n_t, in_=s_n, func=AF.Tanh)

                d = sbp.tile([128, 2, r], F32, tag="de")
                nc.vector.tensor_tensor(out=d, in0=ht, in1=n_t,
                                        op=ALU.subtract)
                e = sbp.tile([128, 2, r], F32, tag="de")
                nc.vector.tensor_tensor(out=e, in0=rz[:, 2:4, :], in1=d,
                                        op=ALU.mult)
                return n_t, e    # h' = n + e, written by caller

            def h_srcs_of(key, ht):
                return [(ht[:, k, :], (lambda kk: (lambda m: wblk(key, kk, m)))(k))
                        for k in range(DC)]

            def gru_level(lvl, ht, r, x_ap, par_srcs):
                # block 0
                rz_srcs = list(par_srcs)
                rz_srcs.append((x_ap, lambda m: wxblk(f"g{lvl}x0", m)))
                g_out = sbp.tile([128, 2, r], adt, tag="nt")
                n_t, e = gru_block(rz_srcs, h_srcs_of(f"g{lvl}h0", ht), ht, r,
                                   {"bhn": f"g{lvl}bhn0"}, True)
                nc.vector.tensor_tensor(out=g_out, in0=n_t, in1=e, op=ALU.add)
                # block 1 (writes state in place)
                n_t, e = gru_block(h_srcs_of(f"g{lvl}i1", g_out),
                                   h_srcs_of(f"g{lvl}h1", ht), ht, r,
                                   {"brz": f"g{lvl}brz1",
                                    "bin": f"g{lvl}bin1",
                                    "bhn": f"g{lvl}bhn1"}, False)
                nc.vector.tensor_tensor(out=ht, in0=n_t, in1=e, op=ALU.add)

            # ---- one time step
            def step_body(i):
                x_t = xp.tile([3, CT], adt)
                nc.sync.dma_start(out=x_t, in_=xs_d[i])

                # ODE phase
                ode_mlp("o1", [h1[:, k, :] for k in range(DC)], h1, C1)
                for _i in range(KSUB):
                    ode_mlp("o2",
                            [bc5(h1, 0), bc5(h1, 1),
                             h2[:, 0, :], h2[:, 1, :]], h2, C2)
                    for _j in range(KSUB):
                        ode_mlp("o3",
                                [bc_chain(h2, 0), bc_chain(h2, 1),
                                 h3[:, 0, :], h3[:, 1, :]], h3, C3)

                # GRU phase
                gru_level(1, h1, C1, x_t[:, 0:C1], [])
                par2 = [(bc5(h1, k),
                         (lambda kk: (lambda m: wblk("g2p0", kk, m)))(k))
                        for k in range(DC)]
                gru_level(2, h2, C2, x_t[:, C1:C1 + C2], par2)
                par3 = [(bc_chain(h2, k),
                         (lambda kk: (lambda m: wblk("g3p0", kk, m)))(k))
                        for k in range(DC)]
                gru_level(3, h3, C3, x_t[:, C1 + C2:CT], par3)

                # output projection
                ps_o = psp4.tile([3, CT], F32, tag="po")
                for c0, r, ht in ((0, C1, h1), (C1, C2, h2), (C1 + C2, C3, h3)):
                    for k in range(DC):
                        mm(ps_o[:, c0:c0 + r], wo_s[:, k * 3:(k + 1) * 3],
                           ht[:, k, :], k == 0, k == DC - 1)
                o_t = outp.tile([3, CT], F32)
                nc.vector.tensor_scalar_add(out=o_t, in0=ps_o,
                                            scalar1=bo_s[:, 0:1])
                nc.sync.dma_start(out=out_d[i], in_=o_t)

            if use_loop is None:
                use_loop = steps > 4
            hints = (mybir.EngineType.PE, mybir.EngineType.DVE,
                     mybir.EngineType.Activation)
            if not use_loop:
                for i in range(steps):
                    step_body(i)
            elif rep > 1:
                with tc.For_i(0, rep) as _r:
                    with tc.For_i(0, steps, hint_engines=hints) as i:
                        step_body(i)
            else:
                with tc.For_i(0, steps, hint_engines=hints) as i:
                    step_body(i)

    nc.compile()
    return nc


# ======================================================================
# host entry point
# ======================================================================
_CACHE = {}
LAST_EXEC_NS = None


def _get_module(steps, offs_key, offs, adt):
    key = (steps, offs_key, str(adt))
    if key not in _CACHE:
        _CACHE[key] = _build(steps, offs, adt)
    return _CACHE[key]


def _prep_core_inputs(x2d, mask, arrays, np_dtype, steps):
    """Per-core xs arrays: [steps, 3, CT], feats (x0, x1, 1), col=j*SEQ+s."""
    xm = (np.asarray(x2d, np.float32) * np.asarray(mask, np.float32))
    # [B, S, P, J, 2] -> per core [S, 2, J, BPC, P] -> [S, 2, J*SEQ]
    in_maps = []
    for c in range(NCORES):
        xc = xm[c * BPC:(c + 1) * BPC, :steps]          # [BPC, S, P, J, 2]
        xc = xc.transpose(1, 4, 3, 0, 2)                # [S, 2, J, BPC, P]
        xc = xc.reshape(steps, 2, CT)
        xs = np.empty((steps, 3, CT), np.float32)
        xs[:, :2] = xc
        xs[:, 2] = 1.0
        m = {"xs": xs.astype(np_dtype)}
        m.update(arrays)
        in_maps.append(m)
    return in_maps


def _assemble_output(results, steps):
    """results[c]['out'] [steps, 3, CT] -> full (B, S, P, J, 3)."""
    out = np.empty((B, steps, P, J, DOUT), np.float32)
    for c in range(NCORES):
        oc = results[c]["out"].reshape(steps, DOUT, J, BPC, P)
        out[c * BPC:(c + 1) * BPC] = oc.transpose(3, 0, 4, 2, 1)
    return out


F32R = mybir.dt.float32r


def kernel(ts, mask, x2d, g1, g2, g3, params, _steps=S, _adt=None,
           _trace=False):
    if _adt is None:
        _adt = F32R
    np_dtype = np.float32 if _adt != BF16 else mybir.dt.np(BF16)
    arrays, offs = _host_pack(params, np_dtype)
    offs["_wcols"] = arrays["wmat"].shape[1]
    offs["_wxcols"] = arrays["wx"].shape[1]
    offs["_bcols"] = arrays["bias"].shape[1]
    nc = _get_module(_steps, "v1", offs, _adt)
    in_maps = _prep_core_inputs(x2d, mask, arrays, np_dtype, _steps)
    res = run_bass_kernel_spmd(nc, in_maps, core_ids=list(range(NCORES)),
                               trace=_trace)
    out = _assemble_output(res.results, _steps)
    global LAST_EXEC_NS
    LAST_EXEC_NS = res.exec_time_ns
    return out
